# revision 4
# baseline (speedup 1.0000x reference)
"""Soft decision-tree layer (depth 4, 16 leaves) on 8 trn2 NeuronCores.

Sharding: 2-way data parallel (token halves) x 4-way expert parallel
(4 leaves per core).  Each core computes, for its 2048 tokens t and its
4 leaves l:  partial[t,:] = bias_partial[t,:] + sum_l path_l(t) * (x[t] @ Wl[l]).
Host sums the 4 expert partials per token half.

The decision GEMM (x@Wd, 0.4% of the FLOPs), the sigmoid path products
and the path-weighted leaf bias  bias_partial = sum_l path_l * bl[l]
are computed on the host in fp32 and uploaded:
  - path columns  [128, 16*4] fp32  (per token-tile, per local leaf)
  - bias_partial  [2048, 1024] fp16 (DMA'd straight into the fp16
    accumulator tiles as their initial value)
This leaves the device program as pure leaf GEMMs (fp16 operands,
fp32 PSUM) + one DVE scalar_tensor_tensor eviction per (tile, leaf,
512-col half):  acc += psum * path_col.

DMA queues: scalar (ACT HWDGE) carries path + x^T, sync (SP HWDGE)
carries leaf weights (leaf 0 split into 512-col halves so the first
chains start sooner) + bias group 1 + outputs, gpsimd (SWDGE) carries
bias group 0 so it lands before the first evictions.
"""

import numpy as np

B, S, H = 2, 2048, 1024
DP, EP = 2, 4            # data-parallel x expert-parallel = 8 cores
T = (B * S) // DP        # 2048 tokens per core
LPC = 16 // EP           # 4 leaves per core
NT = T // 128            # 16 token tiles per core
TG = 2                   # token groups (acc working set = 8 tiles)
TPG = NT // TG           # 8 token tiles per group
KC = H // 128            # 8 contraction chunks

_prog_cache = {}


def _build_program():
    if "nc" in _prog_cache:
        return _prog_cache["nc"]

    from contextlib import ExitStack
    import concourse.bacc as bacc
    import concourse.tile as tile
    import concourse.mybir as mybir

    f32 = mybir.dt.float32
    f16 = mybir.dt.float16
    MULT = mybir.AluOpType.mult
    ADD = mybir.AluOpType.add

    nc = bacc.Bacc("TRN2", target_bir_lowering=False, debug=False, num_devices=8)

    xt_d = nc.dram_tensor("xt", [H, T], f16, kind="ExternalInput").ap()
    wl_d = nc.dram_tensor("wl", [LPC, H, H], f16, kind="ExternalInput").ap()
    path_d = nc.dram_tensor("path", [128, NT * LPC], f32, kind="ExternalInput").ap()
    bias_d = nc.dram_tensor("bias", [T, H], f16, kind="ExternalInput").ap()
    out_d = nc.dram_tensor("out", [T, H], f16, kind="ExternalOutput").ap()

    with tile.TileContext(nc) as tc, ExitStack() as ctx:
        path_pool = ctx.enter_context(tc.tile_pool(name="path", bufs=1))
        xt_pool = ctx.enter_context(tc.tile_pool(name="xt", bufs=1))
        wl_pool = ctx.enter_context(tc.tile_pool(name="wl", bufs=1))
        acc_pool = ctx.enter_context(tc.tile_pool(name="acc", bufs=2))
        ps_pool = ctx.enter_context(tc.tile_pool(name="ps", bufs=8, space="PSUM"))

        # --- accumulators, initialized by DMA with the host-computed
        #     path-weighted bias partial (gpsimd queue: runs in parallel
        #     with the critical xt/wl streams; g0 must land before the
        #     first evictions free the PSUM banks) ---
        acc = {}
        for g in range(TG):
            a = acc_pool.tile([128, TPG * H], f16, tag="acc", name=f"acc{g}")
            if g == 0:
                nc.gpsimd.dma_start(
                    a[:].rearrange("p (t j) -> p t j", j=H),
                    bias_d[g * TPG * 128:(g + 1) * TPG * 128, :]
                    .rearrange("(t p) j -> p t j", p=128))
            acc[g] = a

        # --- scalar HWDGE queue: path columns then resident x^T chunks ---
        pathc = path_pool.tile([128, NT * LPC], f32, tag="path")
        nc.scalar.dma_start(pathc[:], path_d[:, :])
        xt = {}
        for g in range(TG):
            for k in range(KC):
                t_ = xt_pool.tile([128, T // TG], f16, tag=f"xt{k}_{g}",
                                  name=f"xt{k}_{g}")
                nc.scalar.dma_start(
                    t_[:], xt_d[k * 128:(k + 1) * 128,
                                g * (T // TG):(g + 1) * (T // TG)])
                xt[k, g] = t_

        # --- sync HWDGE queue: leaf weights.  Leaf 0 as 512-col halves
        #     (n0 columns first) so the cold-start chains start on 3MB
        #     of DMA instead of 4; bias g1 rides after wl l1. ---
        wl = {}
        for k in range(KC):
            w = wl_pool.tile([128, 512], f16, tag=f"wl0a{k}", name=f"wl0a{k}")
            nc.sync.dma_start(w[:], wl_d[0, k * 128:(k + 1) * 128, 0:512])
            wl[0, 0, k] = w
        for k in range(KC):
            w = wl_pool.tile([128, 512], f16, tag=f"wl0b{k}", name=f"wl0b{k}")
            nc.sync.dma_start(w[:], wl_d[0, k * 128:(k + 1) * 128, 512:1024])
            wl[0, 1, k] = w
        for l in range(1, LPC):
            for k in range(KC):
                w = wl_pool.tile([128, H], f16, tag=f"wl{l}_{k}",
                                 name=f"wl{l}_{k}")
                nc.sync.dma_start(w[:], wl_d[l, k * 128:(k + 1) * 128, :])
                wl[l, k] = w
            if l == 1:
                nc.sync.dma_start(
                    acc[1][:].rearrange("p (t j) -> p t j", j=H),
                    bias_d[TPG * 128:2 * TPG * 128, :]
                    .rearrange("(t p) j -> p t j", p=128))

        def rhs(l, k, half):
            if l == 0:
                return wl[0, half, k][:]
            return wl[l, k][:, half * 512:(half + 1) * 512]

        def evict(g, t, l, half, ps_t):
            col = (g * TPG + t) * LPC + l
            o = t * H + half * 512
            sl = acc[g][:, o:o + 512]
            nc.vector.scalar_tensor_tensor(
                sl, ps_t[:], pathc[:, col:col + 1], sl, op0=MULT, op1=ADD)

        def out_dma(g, t, half):
            r0 = (g * TPG + t) * 128
            o = t * H + half * 512
            nc.sync.dma_start(
                out_d[r0:r0 + 128, half * 512:half * 512 + 512],
                acc[g][:, o:o + 512])

        # ---- group 0, leaf 0, n0 half: k-outer over 8 concurrent
        #      chains so the PE consumes (xt_k, wl0a_k) chunk pairs as
        #      they land ----
        pss = [ps_pool.tile([128, 512], f32, tag="ps", name=f"pa{t}")
               for t in range(TPG)]
        for k in range(KC):
            for t in range(TPG):
                nc.tensor.matmul(
                    pss[t][:], xt[k, 0][:, t * 128:(t + 1) * 128],
                    rhs(0, k, 0), start=(k == 0), stop=(k == KC - 1))
        for t in range(TPG):
            evict(0, t, 0, 0, pss[t])
        # ---- leaf 0, n1 half: t-major on resident data ----
        for t in range(TPG):
            ps = ps_pool.tile([128, 512], f32, tag="ps", name=f"pb{t}")
            for k in range(KC):
                nc.tensor.matmul(
                    ps[:], xt[k, 0][:, t * 128:(t + 1) * 128],
                    rhs(0, k, 1), start=(k == 0), stop=(k == KC - 1))
            evict(0, t, 0, 1, ps)

        # ---- remaining (group, leaf) passes, t-major ----
        for g in range(TG):
            lr = range(1, LPC) if g == 0 else range(LPC)
            for l in lr:
                for t in range(TPG):
                    psl = ps_pool.tile([128, 512], f32, tag="ps",
                                       name=f"pl{g}_{l}_{t}")
                    psr = ps_pool.tile([128, 512], f32, tag="ps",
                                       name=f"pr{g}_{l}_{t}")
                    for k in range(KC):
                        lhsT = xt[k, g][:, t * 128:(t + 1) * 128]
                        nc.tensor.matmul(psl[:], lhsT, rhs(l, k, 0),
                                         start=(k == 0), stop=(k == KC - 1))
                        nc.tensor.matmul(psr[:], lhsT, rhs(l, k, 1),
                                         start=(k == 0), stop=(k == KC - 1))
                    evict(g, t, l, 0, psl)
                    if l == LPC - 1:
                        out_dma(g, t, 0)
                    evict(g, t, l, 1, psr)
                    if l == LPC - 1:
                        out_dma(g, t, 1)

    nc.compile()
    _prog_cache["nc"] = nc
    return nc


def _host_paths(x2, Wd, bd):
    """Torch-faithful path probabilities [Ttot, 16] in fp32."""
    logits = np.einsum('th,nhc->tnc', x2, Wd.astype(np.float32),
                       optimize=True) + bd.astype(np.float32)
    dec = 1.0 / (1.0 + np.exp(-logits))
    path = np.ones((x2.shape[0], 1), dtype=np.float32)
    for level in range(4):
        start = 2 ** level - 1
        lv = dec[:, start:start + 2 ** level, :]
        path = np.concatenate([path * lv[:, :, 0], path * lv[:, :, 1]], axis=-1)
    return path


def _core_inputs(x, Wd, bd, Wl, bl):
    """Build the 8 per-core input dicts (host-side sharding)."""
    x2 = np.ascontiguousarray(x, dtype=np.float32).reshape(B * S, H)
    Wl = np.asarray(Wl, dtype=np.float32)
    bl = np.asarray(bl, dtype=np.float32)
    paths = _host_paths(x2, np.asarray(Wd, np.float32), np.asarray(bd, np.float32))

    xts = [np.ascontiguousarray(x2[d * T:(d + 1) * T].T).astype(np.float16)
           for d in range(DP)]

    in_maps = []
    for c in range(8):
        d, e = c // EP, c % EP
        p = paths[d * T:(d + 1) * T, LPC * e:LPC * (e + 1)]  # [T, 4] f32
        pathc = np.ascontiguousarray(
            p.reshape(NT, 128, LPC).transpose(1, 0, 2).reshape(128, NT * LPC))
        bias = (p @ bl[LPC * e:LPC * (e + 1)]).astype(np.float16)
        in_maps.append({
            "xt": xts[d],
            "wl": np.ascontiguousarray(Wl[LPC * e:LPC * (e + 1)]).astype(
                np.float16),
            "path": pathc,
            "bias": np.ascontiguousarray(bias),
        })
    return in_maps


def kernel(x, Wd, bd, Wl, bl, _want_results=False):
    from concourse import bass_utils

    nc = _build_program()
    in_maps = _core_inputs(x, Wd, bd, Wl, bl)
    res = bass_utils.run_bass_kernel_spmd(nc, in_maps, list(range(8)))

    out = np.empty((DP, T, H), dtype=np.float32)
    for d in range(DP):
        s = np.zeros((T, H), dtype=np.float64)
        for e in range(EP):
            s += res.results[d * EP + e]["out"].astype(np.float64)
        out[d] = s.astype(np.float32)
    out = out.reshape(B, S, H)
    if _want_results:
        return out, res
    return out


# revision 6
# speedup vs baseline: 1.0041x; 1.0041x over previous
"""Soft decision-tree layer (depth 4, 16 leaves) on 8 trn2 NeuronCores.

Sharding: 2-way data parallel (token halves) x 4-way expert parallel
(4 leaves per core).  Each core computes, for its 2048 tokens t and its
4 leaves l:  partial[t,:] = bias_partial[t,:] + sum_l path_l(t) * (x[t] @ Wl[l]).
Host sums the 4 expert partials per token half.

The decision GEMM (x@Wd, 0.4% of the FLOPs), the sigmoid path products
and the path-weighted leaf bias  bias_partial = sum_l path_l * bl[l]
are computed on the host in fp32 and uploaded (path columns fp32,
bias_partial fp16).  The device program is pure leaf GEMMs (fp16
operands, fp32 PSUM) + one DVE op per (tile, leaf, 512-col half):
  leaf 0:   acc  = psum * path_col        (tensor_scalar, PSUM 2x mode)
  leaf 1-3: acc += psum * path_col        (scalar_tensor_tensor)
plus two floating bias adds per tile emitted in the last leaf pass,
so the bias upload is completely off the cold-start critical path.

Cold start: PE pre-warms on a memset tile (~10 matmuls, flips the HAM
clock gate to 2.4GHz before real work), while x^T group-0 chunks race
in split across the scalar-HWDGE and gpsimd-SWDGE queues and leaf-0
weight halves stream on the sync-HWDGE queue.  Leaf 0 runs k-outer
(both 512-col halves) so the PE consumes chunks as they land.
"""

import numpy as np

B, S, H = 2, 2048, 1024
DP, EP = 2, 4            # data-parallel x expert-parallel = 8 cores
T = (B * S) // DP        # 2048 tokens per core
LPC = 16 // EP           # 4 leaves per core
NT = T // 128            # 16 token tiles per core
TG = 2                   # token groups (acc working set = 8 tiles)
TPG = NT // TG           # 8 token tiles per group
KC = H // 128            # 8 contraction chunks

_prog_cache = {}


def _build_program():
    if "nc" in _prog_cache:
        return _prog_cache["nc"]

    from contextlib import ExitStack
    import concourse.bacc as bacc
    import concourse.tile as tile
    import concourse.mybir as mybir

    f32 = mybir.dt.float32
    f16 = mybir.dt.float16
    MULT = mybir.AluOpType.mult
    ADD = mybir.AluOpType.add

    nc = bacc.Bacc("TRN2", target_bir_lowering=False, debug=False, num_devices=8)

    xt_d = nc.dram_tensor("xt", [H, T], f16, kind="ExternalInput").ap()
    wl_d = nc.dram_tensor("wl", [LPC, H, H], f16, kind="ExternalInput").ap()
    path_d = nc.dram_tensor("path", [128, NT * LPC], f32, kind="ExternalInput").ap()
    bias_d = nc.dram_tensor("bias", [T, H], f16, kind="ExternalInput").ap()
    out_d = nc.dram_tensor("out", [T, H], f16, kind="ExternalOutput").ap()

    with tile.TileContext(nc) as tc, ExitStack() as ctx:
        path_pool = ctx.enter_context(tc.tile_pool(name="path", bufs=1))
        xt_pool = ctx.enter_context(tc.tile_pool(name="xt", bufs=1))
        wl_pool = ctx.enter_context(tc.tile_pool(name="wl", bufs=1))
        acc_pool = ctx.enter_context(tc.tile_pool(name="acc", bufs=2))
        bias_pool = ctx.enter_context(tc.tile_pool(name="bias", bufs=2))
        ps_pool = ctx.enter_context(tc.tile_pool(name="ps", bufs=8, space="PSUM"))

        # --- PE pre-warm: ~4.3us of matmuls on a memset tile (no DMA
        #     deps) flips the HAM clock gate to 2.4GHz before real work;
        #     overlaps the framework preamble + first DMAs ---
        warm = path_pool.tile([128, 512], f16, tag="warm")
        nc.vector.memset(warm[:], 0.0)
        wps = ps_pool.tile([128, 512], f32, tag="ps", name="warmps")
        for _ in range(10):
            nc.tensor.matmul(wps[:], warm[:, 0:128], warm[:],
                             start=True, stop=True)

        # --- scalar HWDGE queue: path columns, xt g0 even chunks,
        #     then leaf-1 weights ---
        pathc = path_pool.tile([128, NT * LPC], f32, tag="path")
        nc.scalar.dma_start(pathc[:], path_d[:, :])
        xt = {}

        def load_xt(k, g, eng):
            t_ = xt_pool.tile([128, T // TG], f16, tag=f"xt{k}_{g}",
                              name=f"xt{k}_{g}")
            eng.dma_start(
                t_[:], xt_d[k * 128:(k + 1) * 128,
                            g * (T // TG):(g + 1) * (T // TG)])
            xt[k, g] = t_

        for k in range(0, KC, 2):
            load_xt(k, 0, nc.scalar)
        # --- gpsimd SWDGE queue: xt g0 odd chunks (parallel stream) ---
        for k in range(1, KC, 2):
            load_xt(k, 0, nc.gpsimd)

        # --- sync HWDGE queue: leaf-0 weight halves first (the
        #     cold-start k-outer passes consume them as they land) ---
        wl = {}
        for half in range(2):
            for k in range(KC):
                w = wl_pool.tile([128, 512], f16, tag=f"wl0{half}{k}",
                                 name=f"wl0{half}{k}")
                nc.sync.dma_start(
                    w[:], wl_d[0, k * 128:(k + 1) * 128,
                               half * 512:(half + 1) * 512])
                wl[0, half, k] = w
        # leaf 1 on the scalar queue (it drains right after xt g0 evens)
        for l in range(1, LPC):
            eng = nc.scalar if l == 1 else nc.sync
            for k in range(KC):
                w = wl_pool.tile([128, H], f16, tag=f"wl{l}_{k}",
                                 name=f"wl{l}_{k}")
                eng.dma_start(w[:], wl_d[l, k * 128:(k + 1) * 128, :])
                wl[l, k] = w
        # late, non-critical transfers ride the sync queue
        bias_sb = {}
        for g in range(TG):
            bias_sb[g] = bias_pool.tile([128, TPG * H], f16, tag="bias",
                                        name=f"bias{g}")
        nc.sync.dma_start(
            bias_sb[0][:].rearrange("p (t j) -> p t j", j=H),
            bias_d[0:TPG * 128, :].rearrange("(t p) j -> p t j", p=128))
        for k in range(KC):
            load_xt(k, 1, nc.sync)
        nc.sync.dma_start(
            bias_sb[1][:].rearrange("p (t j) -> p t j", j=H),
            bias_d[TPG * 128:2 * TPG * 128, :].rearrange("(t p) j -> p t j", p=128))

        acc = {}
        for g in range(TG):
            acc[g] = acc_pool.tile([128, TPG * H], f16, tag="acc",
                                   name=f"acc{g}")

        def rhs(l, k, half):
            if l == 0:
                return wl[0, half, k][:]
            return wl[l, k][:, half * 512:(half + 1) * 512]

        def evict(g, t, l, half, ps_t):
            col = (g * TPG + t) * LPC + l
            o = t * H + half * 512
            sl = acc[g][:, o:o + 512]
            if l == 0:
                nc.vector.tensor_scalar(
                    sl, ps_t[:], pathc[:, col:col + 1], None, op0=MULT)
            else:
                nc.vector.scalar_tensor_tensor(
                    sl, ps_t[:], pathc[:, col:col + 1], sl, op0=MULT, op1=ADD)

        def bias_add(g, t, half):
            o = t * H + half * 512
            sl = acc[g][:, o:o + 512]
            nc.vector.tensor_tensor(sl, sl, bias_sb[g][:, o:o + 512], op=ADD)

        def out_dma(g, t, half):
            r0 = (g * TPG + t) * 128
            o = t * H + half * 512
            nc.sync.dma_start(
                out_d[r0:r0 + 128, half * 512:half * 512 + 512],
                acc[g][:, o:o + 512])

        # ---- group 0, leaf 0: k-outer over 8 concurrent chains per
        #      512-col half, consuming (xt_k, wl0_k) pairs as they land
        for half in range(2):
            pss = [ps_pool.tile([128, 512], f32, tag="ps",
                                name=f"p0{half}{t}") for t in range(TPG)]
            for k in range(KC):
                for t in range(TPG):
                    nc.tensor.matmul(
                        pss[t][:], xt[k, 0][:, t * 128:(t + 1) * 128],
                        rhs(0, k, half), start=(k == 0), stop=(k == KC - 1))
            for t in range(TPG):
                evict(0, t, 0, half, pss[t])

        # ---- remaining (group, leaf) passes, t-major ----
        for g in range(TG):
            lr = range(1, LPC) if g == 0 else range(LPC)
            for l in lr:
                for t in range(TPG):
                    psl = ps_pool.tile([128, 512], f32, tag="ps",
                                       name=f"pl{g}_{l}_{t}")
                    psr = ps_pool.tile([128, 512], f32, tag="ps",
                                       name=f"pr{g}_{l}_{t}")
                    for k in range(KC):
                        lhsT = xt[k, g][:, t * 128:(t + 1) * 128]
                        nc.tensor.matmul(psl[:], lhsT, rhs(l, k, 0),
                                         start=(k == 0), stop=(k == KC - 1))
                        nc.tensor.matmul(psr[:], lhsT, rhs(l, k, 1),
                                         start=(k == 0), stop=(k == KC - 1))
                    if l == LPC - 1:
                        # floating bias adds: overlap this tile's chains
                        bias_add(g, t, 0)
                        bias_add(g, t, 1)
                    evict(g, t, l, 0, psl)
                    if l == LPC - 1:
                        out_dma(g, t, 0)
                    evict(g, t, l, 1, psr)
                    if l == LPC - 1:
                        out_dma(g, t, 1)

    nc.compile()
    _prog_cache["nc"] = nc
    return nc


def _host_paths(x2, Wd, bd):
    """Torch-faithful path probabilities [Ttot, 16] in fp32."""
    logits = np.einsum('th,nhc->tnc', x2, Wd.astype(np.float32),
                       optimize=True) + bd.astype(np.float32)
    dec = 1.0 / (1.0 + np.exp(-logits))
    path = np.ones((x2.shape[0], 1), dtype=np.float32)
    for level in range(4):
        start = 2 ** level - 1
        lv = dec[:, start:start + 2 ** level, :]
        path = np.concatenate([path * lv[:, :, 0], path * lv[:, :, 1]], axis=-1)
    return path


def _core_inputs(x, Wd, bd, Wl, bl):
    """Build the 8 per-core input dicts (host-side sharding)."""
    x2 = np.ascontiguousarray(x, dtype=np.float32).reshape(B * S, H)
    Wl = np.asarray(Wl, dtype=np.float32)
    bl = np.asarray(bl, dtype=np.float32)
    paths = _host_paths(x2, np.asarray(Wd, np.float32), np.asarray(bd, np.float32))

    xts = [np.ascontiguousarray(x2[d * T:(d + 1) * T].T).astype(np.float16)
           for d in range(DP)]

    in_maps = []
    for c in range(8):
        d, e = c // EP, c % EP
        p = paths[d * T:(d + 1) * T, LPC * e:LPC * (e + 1)]  # [T, 4] f32
        pathc = np.ascontiguousarray(
            p.reshape(NT, 128, LPC).transpose(1, 0, 2).reshape(128, NT * LPC))
        bias = (p @ bl[LPC * e:LPC * (e + 1)]).astype(np.float16)
        in_maps.append({
            "xt": xts[d],
            "wl": np.ascontiguousarray(Wl[LPC * e:LPC * (e + 1)]).astype(
                np.float16),
            "path": pathc,
            "bias": np.ascontiguousarray(bias),
        })
    return in_maps


def kernel(x, Wd, bd, Wl, bl, _want_results=False):
    from concourse import bass_utils

    nc = _build_program()
    in_maps = _core_inputs(x, Wd, bd, Wl, bl)
    res = bass_utils.run_bass_kernel_spmd(nc, in_maps, list(range(8)))

    out = np.empty((DP, T, H), dtype=np.float32)
    for d in range(DP):
        s = np.zeros((T, H), dtype=np.float64)
        for e in range(EP):
            s += res.results[d * EP + e]["out"].astype(np.float64)
        out[d] = s.astype(np.float32)
    out = out.reshape(B, S, H)
    if _want_results:
        return out, res
    return out


# revision 8
# speedup vs baseline: 1.0188x; 1.0147x over previous
"""Soft decision-tree layer (depth 4, 16 leaves) on 8 trn2 NeuronCores.

Sharding: 2-way data parallel (token halves) x 4-way expert parallel
(4 leaves per core).  Each core computes, for its 2048 tokens t and its
4 leaves l:  partial[t,:] = bias_partial[t,:] + sum_l path_l(t) * (x[t] @ Wl[l]).
Host sums the 4 expert partials per token half.

The decision GEMM (x@Wd, 0.4% of the FLOPs), the sigmoid path products
and the path-weighted leaf bias  bias_partial = sum_l path_l * bl[l]
are computed on the host in fp32 and uploaded (path columns fp32,
bias_partial fp16).  The device program is pure leaf GEMMs (fp16
operands, fp32 PSUM) + one DVE op per (tile, leaf, 512-col half):
  leaf 0:   acc  = psum * path_col        (tensor_scalar, PSUM 2x mode)
  leaf 1-3: acc += psum * path_col        (scalar_tensor_tensor)
plus two floating bias adds per tile emitted in the last leaf pass,
so the bias upload is completely off the cold-start critical path.

Cold start: PE pre-warms on a memset tile (~10 matmuls, flips the HAM
clock gate to 2.4GHz before real work), while x^T group-0 chunks race
in split across the scalar-HWDGE and gpsimd-SWDGE queues and leaf-0
weight halves stream on the sync-HWDGE queue.  Leaf 0 runs k-outer
(both 512-col halves) so the PE consumes chunks as they land.
"""

import numpy as np

B, S, H = 2, 2048, 1024
DP, EP = 2, 4            # data-parallel x expert-parallel = 8 cores
T = (B * S) // DP        # 2048 tokens per core
LPC = 16 // EP           # 4 leaves per core
NT = T // 128            # 16 token tiles per core
TG = 2                   # token groups (acc working set = 8 tiles)
TPG = NT // TG           # 8 token tiles per group
KC = H // 128            # 8 contraction chunks

_prog_cache = {}


def _build_program():
    if "nc" in _prog_cache:
        return _prog_cache["nc"]

    from contextlib import ExitStack
    import concourse.bacc as bacc
    import concourse.tile as tile
    import concourse.mybir as mybir

    f32 = mybir.dt.float32
    f16 = mybir.dt.float16
    MULT = mybir.AluOpType.mult
    ADD = mybir.AluOpType.add

    nc = bacc.Bacc("TRN2", target_bir_lowering=False, debug=False, num_devices=8)

    xt_d = nc.dram_tensor("xt", [H, T], f16, kind="ExternalInput").ap()
    wl_d = nc.dram_tensor("wl", [LPC, H, H], f16, kind="ExternalInput").ap()
    path_d = nc.dram_tensor("path", [128, NT * LPC], f32, kind="ExternalInput").ap()
    bias_d = nc.dram_tensor("bias", [T, H], f16, kind="ExternalInput").ap()
    out_d = nc.dram_tensor("out", [T, H], f16, kind="ExternalOutput").ap()

    with tile.TileContext(nc) as tc, ExitStack() as ctx:
        path_pool = ctx.enter_context(tc.tile_pool(name="path", bufs=1))
        xt_pool = ctx.enter_context(tc.tile_pool(name="xt", bufs=1))
        wl_pool = ctx.enter_context(tc.tile_pool(name="wl", bufs=1))
        acc_pool = ctx.enter_context(tc.tile_pool(name="acc", bufs=2))
        bias_pool = ctx.enter_context(tc.tile_pool(name="bias", bufs=2))
        ps_pool = ctx.enter_context(tc.tile_pool(name="ps", bufs=8, space="PSUM"))

        # --- PE pre-warm: ~4.3us of matmuls on a memset tile (no DMA
        #     deps) flips the HAM clock gate to 2.4GHz before real work;
        #     overlaps the framework preamble + first DMAs ---
        warm = path_pool.tile([128, 512], f16, tag="warm")
        nc.vector.memset(warm[:], 0.0)
        wps = ps_pool.tile([128, 512], f32, tag="ps", name="warmps")
        for _ in range(8):
            nc.tensor.matmul(wps[:], warm[:, 0:128], warm[:],
                             start=True, stop=True)

        xt = {}

        def load_xt(k, g, eng):
            t_ = xt_pool.tile([128, T // TG], f16, tag=f"xt{k}_{g}",
                              name=f"xt{k}_{g}")
            eng.dma_start(
                t_[:], xt_d[k * 128:(k + 1) * 128,
                            g * (T // TG):(g + 1) * (T // TG)])
            xt[k, g] = t_

        # --- sync HWDGE queue: the first xt chunk (split so the first
        #     4 token-tile chains start ~0.5us sooner), then leaf-0
        #     weight halves, which the k-outer passes consume on landing
        xt00 = xt_pool.tile([128, T // TG], f16, tag="xt0_0", name="xt0_0")
        for piece in range(2):
            nc.sync.dma_start(
                xt00[:, piece * 512:(piece + 1) * 512],
                xt_d[0:128, piece * 512:(piece + 1) * 512])
        xt[0, 0] = xt00
        wl = {}
        for half in range(2):
            for k in range(KC):
                w = wl_pool.tile([128, 512], f16, tag=f"wl0{half}{k}",
                                 name=f"wl0{half}{k}")
                nc.sync.dma_start(
                    w[:], wl_d[0, k * 128:(k + 1) * 128,
                               half * 512:(half + 1) * 512])
                wl[0, half, k] = w
        # --- scalar HWDGE queue: remaining xt g0 chunks (the n0 pacer),
        #     path columns, then leaf-1 weights ---
        for k in range(1, KC):
            load_xt(k, 0, nc.scalar)
        pathc = path_pool.tile([128, NT * LPC], f32, tag="path")
        nc.scalar.dma_start(pathc[:], path_d[:, :])
        for l in range(1, LPC):
            eng = nc.scalar if l == 1 else nc.sync
            for k in range(KC):
                w = wl_pool.tile([128, H], f16, tag=f"wl{l}_{k}",
                                 name=f"wl{l}_{k}")
                eng.dma_start(w[:], wl_d[l, k * 128:(k + 1) * 128, :])
                wl[l, k] = w
        # late, non-critical transfers ride the sync queue
        bias_sb = {}
        for g in range(TG):
            bias_sb[g] = bias_pool.tile([128, TPG * H], f16, tag="bias",
                                        name=f"bias{g}")
        nc.sync.dma_start(
            bias_sb[0][:].rearrange("p (t j) -> p t j", j=H),
            bias_d[0:TPG * 128, :].rearrange("(t p) j -> p t j", p=128))
        for k in range(KC):
            load_xt(k, 1, nc.sync)
        nc.sync.dma_start(
            bias_sb[1][:].rearrange("p (t j) -> p t j", j=H),
            bias_d[TPG * 128:2 * TPG * 128, :].rearrange("(t p) j -> p t j", p=128))

        acc = {}
        for g in range(TG):
            acc[g] = acc_pool.tile([128, TPG * H], f16, tag="acc",
                                   name=f"acc{g}")

        def rhs(l, k, half):
            if l == 0:
                return wl[0, half, k][:]
            return wl[l, k][:, half * 512:(half + 1) * 512]

        def evict(g, t, l, half, ps_t):
            col = (g * TPG + t) * LPC + l
            o = t * H + half * 512
            sl = acc[g][:, o:o + 512]
            if l == 0:
                nc.vector.tensor_scalar(
                    sl, ps_t[:], pathc[:, col:col + 1], None, op0=MULT)
            else:
                nc.vector.scalar_tensor_tensor(
                    sl, ps_t[:], pathc[:, col:col + 1], sl, op0=MULT, op1=ADD)

        def bias_add(g, t, half):
            o = t * H + half * 512
            sl = acc[g][:, o:o + 512]
            nc.vector.tensor_tensor(sl, sl, bias_sb[g][:, o:o + 512], op=ADD)

        def out_dma(g, t, half):
            r0 = (g * TPG + t) * 128
            o = t * H + half * 512
            nc.sync.dma_start(
                out_d[r0:r0 + 128, half * 512:half * 512 + 512],
                acc[g][:, o:o + 512])

        # ---- group 0, leaf 0: k-outer over 8 concurrent chains per
        #      512-col half, consuming (xt_k, wl0_k) pairs as they land
        for half in range(2):
            pss = [ps_pool.tile([128, 512], f32, tag="ps",
                                name=f"p0{half}{t}") for t in range(TPG)]
            for k in range(KC):
                for t in range(TPG):
                    nc.tensor.matmul(
                        pss[t][:], xt[k, 0][:, t * 128:(t + 1) * 128],
                        rhs(0, k, half), start=(k == 0), stop=(k == KC - 1))
            for t in range(TPG):
                evict(0, t, 0, half, pss[t])

        # ---- remaining (group, leaf) passes, t-major ----
        for g in range(TG):
            lr = range(1, LPC) if g == 0 else range(LPC)
            for l in lr:
                for t in range(TPG):
                    if l == LPC - 1:
                        # last leaf: split chains so the left eviction,
                        # bias adds and left output DMA overlap the
                        # right chain's matmuls (shorter pipeline tail)
                        bias_add(g, t, 0)
                        bias_add(g, t, 1)
                        for half in range(2):
                            ps = ps_pool.tile([128, 512], f32, tag="ps",
                                              name=f"pq{g}_{t}_{half}")
                            for k in range(KC):
                                nc.tensor.matmul(
                                    ps[:], xt[k, g][:, t * 128:(t + 1) * 128],
                                    rhs(l, k, half),
                                    start=(k == 0), stop=(k == KC - 1))
                            evict(g, t, l, half, ps)
                            out_dma(g, t, half)
                        continue
                    psl = ps_pool.tile([128, 512], f32, tag="ps",
                                       name=f"pl{g}_{l}_{t}")
                    psr = ps_pool.tile([128, 512], f32, tag="ps",
                                       name=f"pr{g}_{l}_{t}")
                    for k in range(KC):
                        lhsT = xt[k, g][:, t * 128:(t + 1) * 128]
                        nc.tensor.matmul(psl[:], lhsT, rhs(l, k, 0),
                                         start=(k == 0), stop=(k == KC - 1))
                        nc.tensor.matmul(psr[:], lhsT, rhs(l, k, 1),
                                         start=(k == 0), stop=(k == KC - 1))
                    evict(g, t, l, 0, psl)
                    evict(g, t, l, 1, psr)

    nc.compile()
    _prog_cache["nc"] = nc
    return nc


def _host_paths(x2, Wd, bd):
    """Torch-faithful path probabilities [Ttot, 16] in fp32."""
    logits = np.einsum('th,nhc->tnc', x2, Wd.astype(np.float32),
                       optimize=True) + bd.astype(np.float32)
    dec = 1.0 / (1.0 + np.exp(-logits))
    path = np.ones((x2.shape[0], 1), dtype=np.float32)
    for level in range(4):
        start = 2 ** level - 1
        lv = dec[:, start:start + 2 ** level, :]
        path = np.concatenate([path * lv[:, :, 0], path * lv[:, :, 1]], axis=-1)
    return path


def _core_inputs(x, Wd, bd, Wl, bl):
    """Build the 8 per-core input dicts (host-side sharding)."""
    x2 = np.ascontiguousarray(x, dtype=np.float32).reshape(B * S, H)
    Wl = np.asarray(Wl, dtype=np.float32)
    bl = np.asarray(bl, dtype=np.float32)
    paths = _host_paths(x2, np.asarray(Wd, np.float32), np.asarray(bd, np.float32))

    xts = [np.ascontiguousarray(x2[d * T:(d + 1) * T].T).astype(np.float16)
           for d in range(DP)]

    in_maps = []
    for c in range(8):
        d, e = c // EP, c % EP
        p = paths[d * T:(d + 1) * T, LPC * e:LPC * (e + 1)]  # [T, 4] f32
        pathc = np.ascontiguousarray(
            p.reshape(NT, 128, LPC).transpose(1, 0, 2).reshape(128, NT * LPC))
        bias = (p @ bl[LPC * e:LPC * (e + 1)]).astype(np.float16)
        in_maps.append({
            "xt": xts[d],
            "wl": np.ascontiguousarray(Wl[LPC * e:LPC * (e + 1)]).astype(
                np.float16),
            "path": pathc,
            "bias": np.ascontiguousarray(bias),
        })
    return in_maps


def kernel(x, Wd, bd, Wl, bl, _want_results=False):
    from concourse import bass_utils

    nc = _build_program()
    in_maps = _core_inputs(x, Wd, bd, Wl, bl)
    res = bass_utils.run_bass_kernel_spmd(nc, in_maps, list(range(8)))

    out = np.empty((DP, T, H), dtype=np.float32)
    for d in range(DP):
        s = np.zeros((T, H), dtype=np.float64)
        for e in range(EP):
            s += res.results[d * EP + e]["out"].astype(np.float64)
        out[d] = s.astype(np.float32)
    out = out.reshape(B, S, H)
    if _want_results:
        return out, res
    return out
